# revision 3
# baseline (speedup 1.0000x reference)
"""Bass/Trainium2 kernel for a 2-layer GAT (PyG GATConv semantics, concat=False,
mean over heads, self-loops, eval-mode dropout) on 8 NeuronCores.

Strategy (vertex 1-D partitioning, dst-sharded):
  - Nodes are sharded by destination across 8 cores (6250 each). Edges live on
    the core owning their destination, sorted by dst, grouped into 128-dst
    windows, tiled into 128-edge tiles (padded; pad edges have dstl=-1 which
    zeroes their one-hot selector column, so they contribute nothing).
  - The host pre-expands per-edge source/dest features into per-tile transposed
    blocks (it knows the graph). On device, each tile does:
      per-edge matmul:  psum[e, 0:8]  = x_src[e]@V_s + x_dst[e]@V_d   (scores)
                        psum[e, 8:..] = x_src[e]@W                    (features)
      Wt = exp(lrelu(scores))         (ScalarE, fused alpha=0.2)
      M  = [Wt | features * Wt_perhead]  (VectorE strided-broadcast mul)
      acc_psum += onehot(dstl).T @ M  (TensorE selector matmul -> per-dst sums)
    Per 128-dst group epilogue normalizes by the summed weights, means heads,
    adds bias (+relu / +log_softmax), and stores the shard.
  - Layer 2 is a second NEFF: layer-1 activations return to the host, which
    expands the per-edge pairs for layer 2 (same edge order / same selectors).

segment-softmax note: reference computes exp(e - segmax)/sum; we compute
exp(e)/sum (scores are O(1) ~ N(0,2), exp is safe in fp32) - identical math.
"""
import math
import numpy as np

import concourse.bass as bass
import concourse.mybir as mybir
import concourse.tile as tile
from concourse import bacc

F32 = mybir.dt.float32
AF = mybir.ActivationFunctionType
OP = mybir.AluOpType

P = 128          # edge-tile size / partition count
DW = 128         # dst-window size (one-hot selector width)

# problem constants (hardcoded per contract; kernel.py must be self-contained)
N = 50000
H = 8
F_IN = 128
HID = 32
OUT = 40
NEG_SLOPE = 0.2
N_CORES = 8


# ---------------------------------------------------------------- host prep

def _prep_edges(edge_index, n, n_cores, dw=DW, p=P):
    """Shard edges by dst, sort by dst, window by dw, tile by p.

    Returns (per-core src/dst int arrays padded to tiles*p, per-core dstl
    [p, T] f32 arrays, tiles_per_group list shared across cores).
    """
    e_src = np.concatenate([edge_index[0], np.arange(n, dtype=np.int64)])
    e_dst = np.concatenate([edge_index[1], np.arange(n, dtype=np.int64)])
    shard = n // n_cores
    groups_per_core = math.ceil(shard / dw)

    core_of = e_dst // shard
    srcs_c, dsts_c = [], []
    counts = np.zeros((n_cores, groups_per_core), dtype=np.int64)
    for c in range(n_cores):
        m = core_of == c
        s, d = e_src[m], e_dst[m]
        order = np.argsort(d, kind="stable")
        s, d = s[order], d[order]
        srcs_c.append(s)
        dsts_c.append(d)
        dl = d - c * shard
        g = dl // dw
        cnt = np.bincount(g, minlength=groups_per_core)
        counts[c] = cnt
    tiles_per_group = [int(math.ceil(counts[:, g].max() / p)) for g in range(groups_per_core)]
    T = int(sum(tiles_per_group))

    src_pad = np.zeros((n_cores, T * p), dtype=np.int64)
    dst_pad = np.zeros((n_cores, T * p), dtype=np.int64)
    dstl = np.full((n_cores, T * p), -1.0, dtype=np.float32)
    for c in range(n_cores):
        s, d = srcs_c[c], dsts_c[c]
        gidx = (d - c * shard) // dw
        start = np.concatenate([[0], np.cumsum(counts[c])])
        off = 0
        for g in range(groups_per_core):
            k = counts[c][g]
            sl = slice(start[g], start[g] + k)
            src_pad[c, off:off + k] = s[sl]
            dst_pad[c, off:off + k] = d[sl]
            dstl[c, off:off + k] = (d[sl] - c * shard - g * dw).astype(np.float32)
            off += tiles_per_group[g] * p
    # dstl layout [p, T] column-per-tile
    dstl_t = dstl.reshape(n_cores, T, p).transpose(0, 2, 1).copy()
    return src_pad, dst_pad, dstl_t, tiles_per_group


def _expand_pairs(x, src_pad, dst_pad, T):
    """Build per-core XPAIR [T, k, 256]: [:, :, 0:128]=x[src].T, 128:256=x[dst].T"""
    k = x.shape[1]
    n_cores = src_pad.shape[0]
    out = np.empty((n_cores, T, k, 2 * P), dtype=np.float32)
    for c in range(n_cores):
        xs = x[src_pad[c]].reshape(T, P, k)
        xd = x[dst_pad[c]].reshape(T, P, k)
        out[c, :, :, 0:P] = xs.transpose(0, 2, 1)
        out[c, :, :, P:2 * P] = xd.transpose(0, 2, 1)
    return out


# ---------------------------------------------------------------- NEFF builder

def build_gat_layer_neff(tiles_per_group, k_in, heads, c_out, W_all, V_s, V_d,
                         bias, shard_rows, final_layer, dw=DW):
    """One GAT layer over this core's dst shard.

    W_all  [k_in, heads*c_out], V_s/V_d [k_in, heads], bias [c_out]
    Input XPAIR [T, k_in, 256] f32, DSTL [128, T] f32.
    Output [shard_rows, c_out] f32 (relu'd hidden, or log_softmax logits).
    """
    T = int(sum(tiles_per_group))
    hc = heads * c_out
    mov = 8 + hc                      # psum cols: scores(8) | features(hc)

    nc = bacc.Bacc(None, target_bir_lowering=False)
    xpair_in = nc.declare_dram_parameter("xpair", [T, k_in, 2 * P], F32, isOutput=False)
    dstl_in = nc.declare_dram_parameter("dstl", [P, T], F32, isOutput=False)
    out_d = nc.declare_dram_parameter("out", [shard_rows, c_out], F32, isOutput=True)

    wa = np.zeros((k_in, mov), dtype=np.float32)
    wa[:, 0:8] = V_s
    wa[:, 8:] = W_all
    wa_c = nc.inline_tensor(wa, name="wa")
    vd_c = nc.inline_tensor(V_d.astype(np.float32), name="vd")
    iota_c = nc.inline_tensor(
        np.tile(np.arange(dw, dtype=np.float32), (P, 1)), name="iota")
    bias_c = nc.inline_tensor(
        np.tile((bias * heads).astype(np.float32), (P, 1)), name="biasx")

    groups = len(tiles_per_group)

    with tile.TileContext(nc) as tc:
        with tc.tile_pool(name="const", bufs=1) as cpool, \
             tc.tile_pool(name="xp", bufs=4) as xpool, \
             tc.tile_pool(name="m", bufs=4) as mpool, \
             tc.tile_pool(name="s0", bufs=4) as s0pool, \
             tc.tile_pool(name="sc", bufs=4) as scpool, \
             tc.tile_pool(name="dl", bufs=2) as dlpool, \
             tc.tile_pool(name="ep", bufs=2) as eppool, \
             tc.tile_pool(name="pp", bufs=3, space="PSUM") as pspool, \
             tc.tile_pool(name="pa", bufs=2, space="PSUM") as papool:

            wa_sb = cpool.tile([k_in, mov], F32)
            nc.sync.dma_start(out=wa_sb[:], in_=wa_c[:])
            vd_sb = cpool.tile([k_in, 8], F32)
            nc.sync.dma_start(out=vd_sb[:], in_=vd_c[:])
            iota_sb = cpool.tile([P, dw], F32)
            nc.sync.dma_start(out=iota_sb[:], in_=iota_c[:])
            bias_sb = cpool.tile([P, c_out], F32)
            nc.sync.dma_start(out=bias_sb[:], in_=bias_c[:])

            t0 = 0
            for g in range(groups):
                ntg = tiles_per_group[g]
                dl_sb = dlpool.tile([P, ntg], F32, tag="dl")
                nc.sync.dma_start(out=dl_sb[:], in_=dstl_in[:, t0:t0 + ntg])
                acc = papool.tile([P, mov], F32, tag="acc")
                for j in range(ntg):
                    t = t0 + j
                    xp = xpool.tile([k_in, 2 * P], F32, tag="xp")
                    nc.sync.dma_start(out=xp[:], in_=xpair_in[t])
                    pp = pspool.tile([P, mov], F32, tag="pp")
                    nc.tensor.matmul(out=pp[:], lhsT=xp[:, 0:P], rhs=wa_sb[:],
                                     start=True, stop=False)
                    nc.tensor.matmul(out=pp[:, 0:8], lhsT=xp[:, P:2 * P],
                                     rhs=vd_sb[:], start=False, stop=True)
                    # scores -> Wt = exp(lrelu(z)); lrelu(z) = max(z, 0.2z)
                    t02 = scpool.tile([P, 8], F32, tag="t02")
                    nc.vector.tensor_scalar_mul(out=t02[:], in0=pp[:, 0:8],
                                                scalar1=NEG_SLOPE)
                    lr = scpool.tile([P, 8], F32, tag="lr")
                    nc.vector.tensor_tensor(out=lr[:], in0=pp[:, 0:8], in1=t02[:],
                                            op=OP.max)
                    m = mpool.tile([P, mov], F32, tag="m")
                    nc.scalar.activation(out=m[:, 0:8], in_=lr[:], func=AF.Exp)
                    # weighted features
                    nc.vector.tensor_tensor(
                        out=m[:, 8:mov].rearrange("p (h c) -> p h c", h=heads),
                        in0=pp[:, 8:mov].rearrange("p (h c) -> p h c", h=heads),
                        in1=m[:, 0:8].unsqueeze(2).to_broadcast([P, heads, c_out]),
                        op=OP.mult)
                    # one-hot selector
                    s0 = s0pool.tile([P, dw], F32, tag="s0")
                    nc.vector.tensor_tensor(
                        out=s0[:], in0=dl_sb[:, j:j + 1].to_broadcast([P, dw]),
                        in1=iota_sb[:], op=OP.is_equal)
                    nc.tensor.matmul(out=acc[:], lhsT=s0[:], rhs=m[:],
                                     start=(j == 0), stop=(j == ntg - 1))
                # ---- group epilogue ----
                rows = min(dw, shard_rows - g * dw)
                sc = eppool.tile([P, 8], F32, tag="sc")
                nc.vector.tensor_scalar_max(out=sc[:], in0=acc[:, 0:8], scalar1=1e-30)
                rec = eppool.tile([P, 8], F32, tag="rec")
                nc.vector.reciprocal(out=rec[:], in_=sc[:])
                pw = eppool.tile([P, hc], F32, tag="pw")
                nc.vector.tensor_tensor(
                    out=pw[:].rearrange("p (h c) -> p h c", h=heads),
                    in0=acc[:, 8:mov].rearrange("p (h c) -> p h c", h=heads),
                    in1=rec[:].unsqueeze(2).to_broadcast([P, heads, c_out]),
                    op=OP.mult)
                # tree-sum heads
                half = hc
                while half > c_out:
                    half //= 2
                    nc.vector.tensor_tensor(out=pw[:, 0:half], in0=pw[:, 0:half],
                                            in1=pw[:, half:2 * half], op=OP.add)
                z = eppool.tile([P, c_out], F32, tag="z")
                nc.vector.tensor_tensor(out=z[:], in0=pw[:, 0:c_out],
                                        in1=bias_sb[:], op=OP.add)
                if not final_layer:
                    # x2 = relu(z/heads) = max(z,0)/heads
                    nc.vector.tensor_scalar(out=z[:], in0=z[:],
                                            scalar1=1.0 / heads, scalar2=0.0,
                                            op0=OP.mult, op1=OP.max)
                else:
                    # z/heads then log_softmax over c_out
                    nc.vector.tensor_scalar_mul(out=z[:], in0=z[:], scalar1=1.0 / heads)
                    mx = eppool.tile([P, 1], F32, tag="mx")
                    nc.vector.tensor_reduce(out=mx[:], in_=z[:],
                                            axis=mybir.AxisListType.X, op=OP.max)
                    nmx = eppool.tile([P, 1], F32, tag="nmx")
                    nc.vector.tensor_scalar_mul(out=nmx[:], in0=mx[:], scalar1=-1.0)
                    ex = eppool.tile([P, c_out], F32, tag="ex")
                    s = eppool.tile([P, 1], F32, tag="s")
                    nc.scalar.activation(out=ex[:], in_=z[:], func=AF.Exp,
                                         bias=nmx[:, 0:1], accum_out=s[:, 0:1])
                    ls = eppool.tile([P, 1], F32, tag="ls")
                    nc.scalar.activation(out=ls[:], in_=s[:], func=AF.Ln)
                    off = eppool.tile([P, 1], F32, tag="off")
                    nc.vector.tensor_tensor(out=off[:], in0=mx[:], in1=ls[:], op=OP.add)
                    nc.vector.tensor_tensor(out=z[:], in0=z[:],
                                            in1=off[:, 0:1].to_broadcast([P, c_out]),
                                            op=OP.subtract)
                nc.sync.dma_start(out=out_d[g * dw:g * dw + rows, :], in_=z[:rows, :])
                t0 += ntg
    nc.compile()
    return nc


# ---------------------------------------------------------------- runner

def _run_spmd(nc, in_maps, n_cores):
    from concourse.bass_utils import run_bass_kernel_spmd
    r = run_bass_kernel_spmd(nc, in_maps, core_ids=list(range(n_cores)), trace=False)
    return r.results


def _layer_weights(W, att_src, att_dst):
    """V_s[f,h] = sum_c W[f, h*C+c]*att_src[h,c]; likewise V_d."""
    heads, c = att_src.shape
    Wr = W.reshape(W.shape[0], heads, c)
    V_s = np.einsum("fhc,hc->fh", Wr, att_src)
    V_d = np.einsum("fhc,hc->fh", Wr, att_dst)
    return V_s.astype(np.float32), V_d.astype(np.float32)


def kernel(x, edge_index, W1, att_src1, att_dst1, b1, W2, att_src2, att_dst2, b2):
    x = np.asarray(x, dtype=np.float32)
    edge_index = np.asarray(edge_index)
    W1 = np.asarray(W1, np.float32); W2 = np.asarray(W2, np.float32)
    att_src1 = np.asarray(att_src1, np.float32); att_dst1 = np.asarray(att_dst1, np.float32)
    att_src2 = np.asarray(att_src2, np.float32); att_dst2 = np.asarray(att_dst2, np.float32)
    b1 = np.asarray(b1, np.float32); b2 = np.asarray(b2, np.float32)

    n = x.shape[0]
    shard = n // N_CORES
    src_pad, dst_pad, dstl_t, tpg = _prep_edges(edge_index, n, N_CORES)
    T = int(sum(tpg))

    V_s1, V_d1 = _layer_weights(W1, att_src1, att_dst1)
    V_s2, V_d2 = _layer_weights(W2, att_src2, att_dst2)

    # ---- layer 1
    nc1 = build_gat_layer_neff(tpg, F_IN, H, HID, W1, V_s1, V_d1, b1,
                               shard, final_layer=False)
    xp1 = _expand_pairs(x, src_pad, dst_pad, T)
    in_maps1 = [{"xpair": xp1[c], "dstl": dstl_t[c]} for c in range(N_CORES)]
    res1 = _run_spmd(nc1, in_maps1, N_CORES)
    x2 = np.concatenate([res1[c]["out"] for c in range(N_CORES)], axis=0)

    # ---- layer 2
    nc2 = build_gat_layer_neff(tpg, HID, H, OUT, W2, V_s2, V_d2, b2,
                               shard, final_layer=True)
    xp2 = _expand_pairs(x2, src_pad, dst_pad, T)
    in_maps2 = [{"xpair": xp2[c], "dstl": dstl_t[c]} for c in range(N_CORES)]
    res2 = _run_spmd(nc2, in_maps2, N_CORES)
    out = np.concatenate([res2[c]["out"] for c in range(N_CORES)], axis=0)
    return out


# revision 6
# speedup vs baseline: 7.8850x; 7.8850x over previous
"""Bass/Trainium2 kernel for a 2-layer GAT (PyG GATConv semantics, concat=False,
mean over heads, self-loops, eval-mode dropout) on 8 NeuronCores.

Strategy (vertex 1-D partitioning, dst-sharded):
  - Nodes sharded by destination across 8 cores (6250 each). Edges live on the
    core owning their destination, sorted by dst, grouped into 128-dst windows,
    tiled into 128-edge tiles (pads have an all-zero selector column -> no-op).
  - Host pre-expands per-edge src/dst features (it knows the graph) into
    column-blocked bf16 uploads, and pre-builds the bf16 one-hot selector
    matrices. Per dst-group the device runs two passes over the group's tiles:
      pass A (scores): psum_sc[e, 8j:8j+8] = x_src[e]@V_s + x_dst[e]@V_d
      batched:         Wt = max(exp(z), exp(0.2 z))     == exp(leakyrelu(z))
      pass B:          psum[e,:] = x_src[e]@W;  m = psum * Wt[head(col)]
                       acc += onehot.T @ [Wt | m]       (selector matmul)
    Epilogue divides by the summed weights, means heads, adds bias
    (+relu, or +log_softmax on the final layer) and stores the shard.
  - Layer 2 is a second NEFF: layer-1 activations return to the host, which
    expands layer-2 pairs (same edge order / same selectors).

segment-softmax: reference computes exp(e - segmax)/sum; we compute
exp(e)/sum (scores are O(1), exp safe in fp32) - identical math.
"""
import math
import numpy as np
import ml_dtypes

import concourse.bass as bass
import concourse.mybir as mybir
import concourse.tile as tile
from concourse import bacc

F32 = mybir.dt.float32
BF16 = mybir.dt.bfloat16
AF = mybir.ActivationFunctionType
OP = mybir.AluOpType
NP_BF16 = ml_dtypes.bfloat16

P = 128          # edge-tile size / partition count
DW = 128         # dst-window size (one-hot selector width)
BLK = 8          # tiles per upload DMA block

N = 50000
H = 8
F_IN = 128
HID = 32
OUT = 40
NEG_SLOPE = 0.2
N_CORES = 8


# ---------------------------------------------------------------- host prep

def _prep_edges(edge_index, n, n_cores, dw=DW, p=P):
    """Shard edges by dst, sort by dst, window by dw, tile by p.

    Returns (src_pad [C, T*p], s0_cols [C, p, T*dw] bf16 one-hot selectors,
    tiles_per_group shared across cores)."""
    e_src = np.concatenate([edge_index[0], np.arange(n, dtype=np.int64)])
    e_dst = np.concatenate([edge_index[1], np.arange(n, dtype=np.int64)])
    shard = n // n_cores
    groups = math.ceil(shard / dw)

    core_of = e_dst // shard
    srcs_c, dsts_c = [], []
    counts = np.zeros((n_cores, groups), dtype=np.int64)
    for c in range(n_cores):
        m = core_of == c
        s, d = e_src[m], e_dst[m]
        order = np.argsort(d, kind="stable")
        s, d = s[order], d[order]
        srcs_c.append(s)
        dsts_c.append(d)
        counts[c] = np.bincount((d - c * shard) // dw, minlength=groups)
    tiles_per_group = [int(math.ceil(counts[:, g].max() / p)) for g in range(groups)]
    T = int(sum(tiles_per_group))

    src_pad = np.zeros((n_cores, T * p), dtype=np.int64)
    dst_pad = np.zeros((n_cores, T * p), dtype=np.int64)
    dstl = np.full((n_cores, T * p), -1.0, dtype=np.float32)
    for c in range(n_cores):
        s, d = srcs_c[c], dsts_c[c]
        start = np.concatenate([[0], np.cumsum(counts[c])])
        off = 0
        for g in range(groups):
            k = int(counts[c][g])
            sl = slice(start[g], start[g] + k)
            src_pad[c, off:off + k] = s[sl]
            dst_pad[c, off:off + k] = d[sl]
            dstl[c, off:off + k] = (d[sl] - c * shard - g * dw).astype(np.float32)
            off += tiles_per_group[g] * p
    # one-hot selectors, column-blocked: s0_cols[c][e, T*dw] bf16
    oh = (dstl.reshape(n_cores, T, p)[:, :, :, None] ==
          np.arange(dw, dtype=np.float32)[None, None, None, :])
    s0_cols = np.ascontiguousarray(
        oh.astype(NP_BF16).transpose(0, 2, 1, 3).reshape(n_cores, p, T * dw))
    return src_pad, dst_pad, s0_cols, tiles_per_group


def _expand_pairs_cols(x_bf, src_pad, dst_pad, T):
    """Column-blocked per-edge pairs: out[c][k, T*256] bf16,
    cols [256t:256t+128]=x[src].T, [256t+128:256t+256]=x[dst].T"""
    k = x_bf.shape[1]
    n_cores = src_pad.shape[0]
    out = np.empty((n_cores, k, T, 2 * P), dtype=NP_BF16)
    for c in range(n_cores):
        out[c, :, :, 0:P] = x_bf[src_pad[c]].reshape(T, P, k).transpose(2, 0, 1)
        out[c, :, :, P:2 * P] = x_bf[dst_pad[c]].reshape(T, P, k).transpose(2, 0, 1)
    return np.ascontiguousarray(out.reshape(n_cores, k, T * 2 * P))


# ---------------------------------------------------------------- NEFF builder

def build_gat_layer_neff(tiles_per_group, k_in, heads, c_out, W_all, V_s, V_d,
                         bias, shard_rows, final_layer, dw=DW):
    T = int(sum(tiles_per_group))
    hc = heads * c_out

    nc = bacc.Bacc(None, target_bir_lowering=False)
    xp_in = nc.declare_dram_parameter("xpair", [k_in, T * 2 * P], BF16, isOutput=False)
    s0_in = nc.declare_dram_parameter("s0", [P, T * dw], BF16, isOutput=False)
    out_d = nc.declare_dram_parameter("out", [shard_rows, c_out], F32, isOutput=True)

    w_c = nc.inline_tensor(W_all.astype(NP_BF16), name="w")
    vs_c = nc.inline_tensor(V_s.astype(NP_BF16), name="vs")
    vd_c = nc.inline_tensor(V_d.astype(NP_BF16), name="vd")
    bias_c = nc.inline_tensor(
        np.tile((bias * heads).astype(np.float32), (P, 1)), name="biasx")

    groups = len(tiles_per_group)
    max_ntg = max(tiles_per_group)

    with tile.TileContext(nc) as tc:
        with tc.tile_pool(name="const", bufs=1) as cpool, \
             tc.tile_pool(name="xb", bufs=3) as xbpool, \
             tc.tile_pool(name="sb", bufs=3) as sbpool, \
             tc.tile_pool(name="m", bufs=4) as mpool, \
             tc.tile_pool(name="wt", bufs=2) as wtpool, \
             tc.tile_pool(name="ep", bufs=2) as eppool, \
             tc.tile_pool(name="pp", bufs=3, space="PSUM") as pppool, \
             tc.tile_pool(name="sc", bufs=2, space="PSUM") as scpool, \
             tc.tile_pool(name="pa", bufs=2, space="PSUM") as papool:

            w_sb = cpool.tile([k_in, hc], BF16)
            nc.sync.dma_start(out=w_sb[:], in_=w_c[:])
            vs_sb = cpool.tile([k_in, 8], BF16)
            nc.sync.dma_start(out=vs_sb[:], in_=vs_c[:])
            vd_sb = cpool.tile([k_in, 8], BF16)
            nc.sync.dma_start(out=vd_sb[:], in_=vd_c[:])
            bias_sb = cpool.tile([P, c_out], F32)
            nc.sync.dma_start(out=bias_sb[:], in_=bias_c[:])

            t0 = 0
            for g in range(groups):
                ntg = tiles_per_group[g]
                # upload blocks for this group
                xbs, s0s = [], []
                for b0 in range(0, ntg, BLK):
                    nb = min(BLK, ntg - b0)
                    xb = xbpool.tile([k_in, BLK * 2 * P], BF16, tag="xb")
                    nc.sync.dma_start(
                        out=xb[:, 0:nb * 2 * P],
                        in_=xp_in[:, (t0 + b0) * 2 * P:(t0 + b0 + nb) * 2 * P])
                    s0b = sbpool.tile([P, BLK * dw], BF16, tag="s0b")
                    nc.sync.dma_start(
                        out=s0b[:, 0:nb * dw],
                        in_=s0_in[:, (t0 + b0) * dw:(t0 + b0 + nb) * dw])
                    xbs.append(xb)
                    s0s.append(s0b)

                def xsrc(j):
                    return xbs[j // BLK][:, (j % BLK) * 2 * P:(j % BLK) * 2 * P + P]

                def xdst(j):
                    return xbs[j // BLK][:, (j % BLK) * 2 * P + P:(j % BLK + 1) * 2 * P]

                def s0(j):
                    return s0s[j // BLK][:, (j % BLK) * dw:(j % BLK + 1) * dw]

                # pass A: scores
                sc_ps = scpool.tile([P, 8 * max_ntg], F32, tag="scp")
                for j in range(ntg):
                    nc.tensor.matmul(out=sc_ps[:, 8 * j:8 * j + 8], lhsT=xsrc(j),
                                     rhs=vs_sb[:], start=True, stop=False)
                    nc.tensor.matmul(out=sc_ps[:, 8 * j:8 * j + 8], lhsT=xdst(j),
                                     rhs=vd_sb[:], start=False, stop=True)
                # batched Wt = max(exp(z), exp(0.2 z))  [== exp(leakyrelu(z))]
                e1 = wtpool.tile([P, 8 * max_ntg], BF16, tag="e1")
                nc.scalar.activation(out=e1[:, 0:8 * ntg], in_=sc_ps[:, 0:8 * ntg],
                                     func=AF.Exp)
                e2 = wtpool.tile([P, 8 * max_ntg], BF16, tag="e2")
                nc.scalar.activation(out=e2[:, 0:8 * ntg], in_=sc_ps[:, 0:8 * ntg],
                                     func=AF.Exp, scale=NEG_SLOPE)
                wtm = wtpool.tile([P, 8 * max_ntg], BF16, tag="wtm")
                nc.vector.tensor_tensor(out=wtm[:, 0:8 * ntg], in0=e1[:, 0:8 * ntg],
                                        in1=e2[:, 0:8 * ntg], op=OP.max)

                # pass B: features, weighting, selector accumulate
                acc = papool.tile([P, 8 + hc], F32, tag="acc")
                for j in range(ntg):
                    pp = pppool.tile([P, hc], F32, tag="pp")
                    nc.tensor.matmul(out=pp[:], lhsT=xsrc(j), rhs=w_sb[:],
                                     start=True, stop=True)
                    m = mpool.tile([P, 8 + hc], BF16, tag="m")
                    nc.vector.tensor_copy(out=m[:, 0:8], in_=wtm[:, 8 * j:8 * j + 8])
                    nc.vector.tensor_tensor(
                        out=m[:, 8:8 + hc].rearrange("p (h c) -> p h c", h=heads),
                        in0=pp[:].rearrange("p (h c) -> p h c", h=heads),
                        in1=wtm[:, 8 * j:8 * j + 8].unsqueeze(2)
                            .to_broadcast([P, heads, c_out]),
                        op=OP.mult)
                    nc.tensor.matmul(out=acc[:], lhsT=s0(j), rhs=m[:],
                                     start=(j == 0), stop=(j == ntg - 1))

                # ---- group epilogue ----
                rows = min(dw, shard_rows - g * dw)
                sc = eppool.tile([P, 8], F32, tag="sc")
                nc.vector.tensor_scalar_max(out=sc[:], in0=acc[:, 0:8], scalar1=1e-30)
                rec = eppool.tile([P, 8], F32, tag="rec")
                nc.vector.reciprocal(out=rec[:], in_=sc[:])
                pw = eppool.tile([P, hc], F32, tag="pw")
                nc.vector.tensor_tensor(
                    out=pw[:].rearrange("p (h c) -> p h c", h=heads),
                    in0=acc[:, 8:8 + hc].rearrange("p (h c) -> p h c", h=heads),
                    in1=rec[:].unsqueeze(2).to_broadcast([P, heads, c_out]),
                    op=OP.mult)
                half = hc
                while half > c_out:
                    half //= 2
                    nc.vector.tensor_tensor(out=pw[:, 0:half], in0=pw[:, 0:half],
                                            in1=pw[:, half:2 * half], op=OP.add)
                z = eppool.tile([P, c_out], F32, tag="z")
                nc.vector.tensor_tensor(out=z[:], in0=pw[:, 0:c_out],
                                        in1=bias_sb[:], op=OP.add)
                if not final_layer:
                    nc.vector.tensor_scalar(out=z[:], in0=z[:],
                                            scalar1=1.0 / heads, scalar2=0.0,
                                            op0=OP.mult, op1=OP.max)
                else:
                    nc.vector.tensor_scalar_mul(out=z[:], in0=z[:], scalar1=1.0 / heads)
                    mx = eppool.tile([P, 1], F32, tag="mx")
                    nc.vector.tensor_reduce(out=mx[:], in_=z[:],
                                            axis=mybir.AxisListType.X, op=OP.max)
                    nmx = eppool.tile([P, 1], F32, tag="nmx")
                    nc.vector.tensor_scalar_mul(out=nmx[:], in0=mx[:], scalar1=-1.0)
                    ex = eppool.tile([P, c_out], F32, tag="ex")
                    s = eppool.tile([P, 1], F32, tag="s")
                    nc.scalar.activation(out=ex[:], in_=z[:], func=AF.Exp,
                                         bias=nmx[:, 0:1], accum_out=s[:, 0:1])
                    ls = eppool.tile([P, 1], F32, tag="ls")
                    nc.scalar.activation(out=ls[:], in_=s[:], func=AF.Ln)
                    off = eppool.tile([P, 1], F32, tag="off")
                    nc.vector.tensor_tensor(out=off[:], in0=mx[:], in1=ls[:], op=OP.add)
                    nc.vector.tensor_tensor(out=z[:], in0=z[:],
                                            in1=off[:, 0:1].to_broadcast([P, c_out]),
                                            op=OP.subtract)
                nc.sync.dma_start(out=out_d[g * dw:g * dw + rows, :], in_=z[:rows, :])
                t0 += ntg
    nc.compile()
    return nc


# ---------------------------------------------------------------- runner

def _run_spmd(nc, in_maps, n_cores):
    from concourse.bass_utils import run_bass_kernel_spmd
    r = run_bass_kernel_spmd(nc, in_maps, core_ids=list(range(n_cores)), trace=False)
    return r.results


def _layer_weights(W, att_src, att_dst):
    heads, c = att_src.shape
    Wr = W.reshape(W.shape[0], heads, c)
    V_s = np.einsum("fhc,hc->fh", Wr, att_src)
    V_d = np.einsum("fhc,hc->fh", Wr, att_dst)
    return V_s.astype(np.float32), V_d.astype(np.float32)


def kernel(x, edge_index, W1, att_src1, att_dst1, b1, W2, att_src2, att_dst2, b2):
    x = np.asarray(x, dtype=np.float32)
    edge_index = np.asarray(edge_index)
    W1 = np.asarray(W1, np.float32); W2 = np.asarray(W2, np.float32)
    att_src1 = np.asarray(att_src1, np.float32); att_dst1 = np.asarray(att_dst1, np.float32)
    att_src2 = np.asarray(att_src2, np.float32); att_dst2 = np.asarray(att_dst2, np.float32)
    b1 = np.asarray(b1, np.float32); b2 = np.asarray(b2, np.float32)

    n = x.shape[0]
    shard = n // N_CORES
    src_pad, dst_pad, s0_cols, tpg = _prep_edges(edge_index, n, N_CORES)
    T = int(sum(tpg))

    V_s1, V_d1 = _layer_weights(W1, att_src1, att_dst1)
    V_s2, V_d2 = _layer_weights(W2, att_src2, att_dst2)

    nc1 = build_gat_layer_neff(tpg, F_IN, H, HID, W1, V_s1, V_d1, b1,
                               shard, final_layer=False)
    xp1 = _expand_pairs_cols(x.astype(NP_BF16), src_pad, dst_pad, T)
    in1 = [{"xpair": xp1[c], "s0": s0_cols[c]} for c in range(N_CORES)]
    res1 = _run_spmd(nc1, in1, N_CORES)
    x2 = np.concatenate([res1[c]["out"] for c in range(N_CORES)], axis=0)

    nc2 = build_gat_layer_neff(tpg, HID, H, OUT, W2, V_s2, V_d2, b2,
                               shard, final_layer=True)
    xp2 = _expand_pairs_cols(x2.astype(NP_BF16), src_pad, dst_pad, T)
    in2 = [{"xpair": xp2[c], "s0": s0_cols[c]} for c in range(N_CORES)]
    res2 = _run_spmd(nc2, in2, N_CORES)
    return np.concatenate([res2[c]["out"] for c in range(N_CORES)], axis=0)
